# revision 4
# baseline (speedup 1.0000x reference)
"""MoE MLP (top-2 of 8 experts) on 8 Trainium2 NeuronCores.

Strategy: expert parallelism. Each of the 8 cores owns one expert.
Host-side (inside kernel()): route tokens to experts, gather each
expert's tokens into a dense padded [H, T_cap] activation block
(transposed so it is directly usable as the matmul moving operand),
and ship it with that expert's weights to its core. Each core runs
two dense matmuls (down -> relu -> up) entirely out of SBUF and
scales rows by the per-token routing weight. Host-side combine is a
pure gather-add: every token has exactly K=2 expert contributions.

Device compute per core (bf16, fp32 PSUM accumulation):
  hidT[D, T] = down[H, D]^T @ xT[H, T]   (relu)
  y[T, H]    = hidT[D, T]^T @ up[D, H]   (* routing weight per row)
"""

import os
import sys

import numpy as np

for _p in ("/opt/trn_rl_repo", "/root/.axon_site/_ro/trn_rl_repo"):
    if os.path.isdir(_p) and _p not in sys.path:
        sys.path.append(_p)

import ml_dtypes

import concourse.bass as bass
import concourse.mybir as mybir
from concourse import bacc
from concourse.bass_utils import run_bass_kernel_spmd
from concourse.tile import TileContext

BF16 = ml_dtypes.bfloat16

B, S, H, E, K, D = 1, 4096, 1024, 8, 2, 2048
N = B * S
P = 128
KH = H // P   # 8 contraction tiles for the down matmul
KD = D // P   # 16 contraction tiles for the up matmul
NCORES = 8

# Exposed for test harness introspection (exec_time_ns etc).
LAST_RESULT = None


def _build_bass(t_cap: int) -> bass.Bass:
    """One expert's MLP: y[T,H] = w * (relu(x @ down) @ up)."""
    bf16 = mybir.dt.bfloat16
    f32 = mybir.dt.float32
    relu = mybir.ActivationFunctionType.Relu
    copy = mybir.ActivationFunctionType.Copy
    n_mt = t_cap // P

    nc = bacc.Bacc()
    xT = nc.dram_tensor("xT", [H, t_cap], bf16, kind="ExternalInput")
    dw = nc.dram_tensor("dw", [H, D], bf16, kind="ExternalInput")
    up = nc.dram_tensor("up", [D, H], bf16, kind="ExternalInput")
    wv = nc.dram_tensor("wv", [t_cap, 1], f32, kind="ExternalInput")
    y = nc.dram_tensor("y", [t_cap, H], f32, kind="ExternalOutput")

    with TileContext(nc) as tc:
        with (
            tc.tile_pool(name="const", bufs=1) as const,
            tc.tile_pool(name="psum", bufs=8, space="PSUM") as psum,
            tc.tile_pool(name="outp", bufs=4) as outp,
        ):
            dw_sb = const.tile([P, KH, D], bf16)
            xT_sb = const.tile([P, KH, t_cap], bf16)
            up_sb = const.tile([P, KD, H], bf16)
            hid_sb = const.tile([P, KD, t_cap], bf16)
            wv_sb = const.tile([P, n_mt, 1], f32)

            nc.sync.dma_start(dw_sb[:], dw[:, :].rearrange("(k p) d -> p k d", p=P))
            nc.sync.dma_start(xT_sb[:], xT[:, :].rearrange("(k p) t -> p k t", p=P))
            nc.sync.dma_start(wv_sb[:], wv[:, :].rearrange("(m p) o -> p m o", p=P))
            nc.sync.dma_start(up_sb[:], up[:, :].rearrange("(k p) h -> p k h", p=P))

            # mm1: hidT[D, T] = down^T @ xT, relu, in D-tiles of 128 rows.
            n_off = 0
            while n_off < t_cap:
                n_size = min(512, t_cap - n_off)
                for m in range(KD):
                    ps = psum.tile([P, n_size], f32, tag="ps")
                    for k in range(KH):
                        nc.tensor.matmul(
                            ps[:],
                            dw_sb[:, k, m * P : (m + 1) * P],
                            xT_sb[:, k, n_off : n_off + n_size],
                            start=(k == 0),
                            stop=(k == KH - 1),
                        )
                    nc.scalar.activation(
                        hid_sb[:, m, n_off : n_off + n_size], ps[:], relu
                    )
                n_off += n_size

            # mm2: y[T, H] = hidT^T @ up, scaled per token row.
            for mt in range(n_mt):
                for nh in range(H // 512):
                    ps = psum.tile([P, 512], f32, tag="ps")
                    for k in range(KD):
                        nc.tensor.matmul(
                            ps[:],
                            hid_sb[:, k, mt * P : (mt + 1) * P],
                            up_sb[:, k, nh * 512 : (nh + 1) * 512],
                            start=(k == 0),
                            stop=(k == KD - 1),
                        )
                    yt = outp.tile([P, 512], f32, tag="yt")
                    nc.scalar.activation(yt[:], ps[:], copy, scale=wv_sb[:, mt, :])
                    nc.sync.dma_start(
                        y[mt * P : (mt + 1) * P, nh * 512 : (nh + 1) * 512], yt[:]
                    )
    nc.compile()
    return nc


def _route(expert_weights, chosen_expert_indices, attention_mask):
    """Host-side routing. Returns (token ids per expert, weights per
    expert, padded positions per (token, k) pair, T_cap)."""
    idx = np.asarray(chosen_expert_indices).reshape(N, K).astype(np.int64)
    wts = np.asarray(expert_weights).reshape(N, K).astype(np.float32)
    mask = np.asarray(attention_mask).reshape(N, 1).astype(np.float32)
    wts = wts * mask

    flat_e = idx.reshape(-1)  # [N*K]
    order = np.argsort(flat_e, kind="stable")
    counts = np.bincount(flat_e, minlength=E)
    offsets = np.zeros(E + 1, np.int64)
    np.cumsum(counts, out=offsets[1:])
    t_cap = max(P, int(-(-counts.max() // P) * P))

    rank = np.empty(N * K, np.int64)
    rank[order] = np.arange(N * K) - np.repeat(offsets[:-1], counts)
    pad_pos = flat_e * t_cap + rank  # row of pair (n,k) in concat output

    toks = [order[offsets[e] : offsets[e + 1]] // K for e in range(E)]
    w_e = [wts.reshape(-1)[order[offsets[e] : offsets[e + 1]]] for e in range(E)]
    return toks, w_e, pad_pos, t_cap


def kernel(x, attention_mask, expert_weights, chosen_expert_indices, down_proj, up_proj):
    global LAST_RESULT
    xt = np.asarray(x, dtype=np.float32).reshape(N, H)
    toks, w_e, pad_pos, t_cap = _route(
        expert_weights, chosen_expert_indices, attention_mask
    )

    xT_full = np.ascontiguousarray(xt.T)  # [H, N]
    down = np.asarray(down_proj, dtype=np.float32)
    up = np.asarray(up_proj, dtype=np.float32)

    in_maps = []
    for e in range(E):
        t_e = len(toks[e])
        xTg = np.zeros((H, t_cap), dtype=BF16)
        xTg[:, :t_e] = xT_full[:, toks[e]].astype(BF16)
        wv = np.zeros((t_cap, 1), dtype=np.float32)
        wv[:t_e, 0] = w_e[e]
        in_maps.append(
            {
                "xT": xTg,
                "dw": down[e].astype(BF16),
                "up": up[e].astype(BF16),
                "wv": wv,
            }
        )

    nc = _build_bass(t_cap)
    res = run_bass_kernel_spmd(nc, in_maps, core_ids=list(range(NCORES)))
    LAST_RESULT = res

    y_all = np.concatenate([res.results[e]["y"] for e in range(E)], axis=0)
    contrib = y_all[pad_pos]  # [N*K, H]
    out = xt + contrib[0::2] + contrib[1::2]
    return out.reshape(B, S, H).astype(np.float32)


# revision 9
# speedup vs baseline: 1.0373x; 1.0373x over previous
"""MoE MLP (top-2 of 8 experts) on 8 Trainium2 NeuronCores.

Strategy: expert parallelism. Each of the 8 cores owns one expert.
Host-side (inside kernel()): route tokens to experts, gather each
expert's tokens into a dense padded [H, T_cap] activation block
(transposed so it is directly usable as the matmul moving operand),
and ship it with that expert's weights to its core. Each core runs
two dense matmuls (down -> relu -> up) entirely out of SBUF and
scales rows by the per-token routing weight. Host-side combine is a
pure gather-add: every token has exactly K=2 expert contributions.

Device compute per core (bf16, fp32 PSUM accumulation):
  hidT[D, T] = down[H, D]^T @ xT[H, T]   (relu)
  y[T, H]    = hidT[D, T]^T @ up[D, H]   (* routing weight per row)
"""

import os
import sys

import numpy as np

for _p in ("/opt/trn_rl_repo", "/root/.axon_site/_ro/trn_rl_repo"):
    if os.path.isdir(_p) and _p not in sys.path:
        sys.path.append(_p)

import ml_dtypes

import concourse.bass as bass
import concourse.mybir as mybir
from concourse import bacc
from concourse.bass_utils import run_bass_kernel_spmd
from concourse.tile import TileContext

BF16 = ml_dtypes.bfloat16

B, S, H, E, K, D = 1, 4096, 1024, 8, 2, 2048
N = B * S
P = 128
KH = H // P   # 8 contraction tiles for the down matmul
KD = D // P   # 16 contraction tiles for the up matmul
NCORES = 8

# Exposed for test harness introspection (exec_time_ns etc).
LAST_RESULT = None


def _build_bass(t_cap: int) -> bass.Bass:
    """One expert's MLP: y[T,H] = w * (relu(x @ down) @ up)."""
    bf16 = mybir.dt.bfloat16
    f32 = mybir.dt.float32
    n_mt = t_cap // P

    nc = bacc.Bacc()
    xT = nc.dram_tensor("xT", [H, t_cap], bf16, kind="ExternalInput")
    dw = nc.dram_tensor("dw", [H, D], bf16, kind="ExternalInput")
    up = nc.dram_tensor("up", [D, H], bf16, kind="ExternalInput")
    wv = nc.dram_tensor("wv", [t_cap, 1], f32, kind="ExternalInput")
    y = nc.dram_tensor("y", [t_cap, H], f32, kind="ExternalOutput")

    with TileContext(nc) as tc:
        with (
            tc.tile_pool(name="const", bufs=1) as const,
            tc.tile_pool(name="psum", bufs=1, space="PSUM") as psum,
            tc.tile_pool(name="outp", bufs=4) as outp,
        ):
            dw_sb = const.tile([P, KH, D], bf16)
            xT_sb = const.tile([P, KH, t_cap], bf16)
            up_sb = const.tile([P, KD, H], bf16)
            hid_sb = const.tile([P, KD, t_cap], bf16)
            wv_sb = const.tile([P, n_mt, 1], f32)

            # Per-chunk loads, interleaved so contraction chunk k of both
            # mm1 operands lands together: the k-outer matmul loop below
            # can start as soon as chunk 0 arrives instead of waiting for
            # the full 6.5MB. dw on the SP HWDGE ring, xT on the ACT ring
            # (two rings drain in parallel); up/wv via SWDGE (not on the
            # critical path).
            for k in range(KH):
                nc.sync.dma_start(dw_sb[:, k, :], dw[k * P : (k + 1) * P, :])
                nc.scalar.dma_start(xT_sb[:, k, :], xT[k * P : (k + 1) * P, :])
            for k in range(KD):
                nc.gpsimd.dma_start(up_sb[:, k, :], up[k * P : (k + 1) * P, :])
            nc.gpsimd.dma_start(
                wv_sb[:], wv[:, :].rearrange("(m p) o -> p m o", p=P)
            )

            # mm1: hidT[D, T] = down^T @ xT with relu, k-outermost over 8
            # concurrent PSUM accumulation groups so each matmul only
            # depends on input chunk k.
            n_off = 0
            while n_off < t_cap:
                n_size = min(512, t_cap - n_off)
                for mh in range(KD // 8):
                    pss = [
                        psum.tile([P, n_size], f32, tag=f"ps{m}", name=f"ps{m}")
                        for m in range(8)
                    ]
                    for k in range(KH):
                        for m in range(8):
                            md = mh * 8 + m
                            nc.tensor.matmul(
                                pss[m][:],
                                dw_sb[:, k, md * P : (md + 1) * P],
                                xT_sb[:, k, n_off : n_off + n_size],
                                start=(k == 0),
                                stop=(k == KH - 1),
                            )
                    for m in range(8):
                        md = mh * 8 + m
                        nc.vector.tensor_scalar_max(
                            hid_sb[:, md, n_off : n_off + n_size], pss[m][:], 0.0
                        )
                n_off += n_size

            # mm2: y[T, H] = hidT^T @ up, scaled per token row.
            for mt in range(n_mt):
                for nh in range(H // 512):
                    ps = psum.tile([P, 512], f32, tag=f"ps{(2 * mt + nh) % 8}")
                    for k in range(KD):
                        nc.tensor.matmul(
                            ps[:],
                            hid_sb[:, k, mt * P : (mt + 1) * P],
                            up_sb[:, k, nh * 512 : (nh + 1) * 512],
                            start=(k == 0),
                            stop=(k == KD - 1),
                        )
                    yt = outp.tile([P, 512], f32, tag="yt")
                    nc.vector.tensor_scalar_mul(yt[:], ps[:], wv_sb[:, mt, :])
                    nc.sync.dma_start(
                        y[mt * P : (mt + 1) * P, nh * 512 : (nh + 1) * 512], yt[:]
                    )
    nc.compile()
    return nc


def _route(expert_weights, chosen_expert_indices, attention_mask):
    """Host-side routing. Returns (token ids per expert, weights per
    expert, padded positions per (token, k) pair, T_cap)."""
    idx = np.asarray(chosen_expert_indices).reshape(N, K).astype(np.int64)
    wts = np.asarray(expert_weights).reshape(N, K).astype(np.float32)
    mask = np.asarray(attention_mask).reshape(N, 1).astype(np.float32)
    wts = wts * mask

    flat_e = idx.reshape(-1)  # [N*K]
    order = np.argsort(flat_e, kind="stable")
    counts = np.bincount(flat_e, minlength=E)
    offsets = np.zeros(E + 1, np.int64)
    np.cumsum(counts, out=offsets[1:])
    t_cap = max(P, int(-(-counts.max() // P) * P))

    rank = np.empty(N * K, np.int64)
    rank[order] = np.arange(N * K) - np.repeat(offsets[:-1], counts)
    pad_pos = flat_e * t_cap + rank  # row of pair (n,k) in concat output

    toks = [order[offsets[e] : offsets[e + 1]] // K for e in range(E)]
    w_e = [wts.reshape(-1)[order[offsets[e] : offsets[e + 1]]] for e in range(E)]
    return toks, w_e, pad_pos, t_cap


def kernel(x, attention_mask, expert_weights, chosen_expert_indices, down_proj, up_proj):
    global LAST_RESULT
    xt = np.asarray(x, dtype=np.float32).reshape(N, H)
    toks, w_e, pad_pos, t_cap = _route(
        expert_weights, chosen_expert_indices, attention_mask
    )

    xT_full = np.ascontiguousarray(xt.T)  # [H, N]
    down = np.asarray(down_proj, dtype=np.float32)
    up = np.asarray(up_proj, dtype=np.float32)

    in_maps = []
    for e in range(E):
        t_e = len(toks[e])
        xTg = np.zeros((H, t_cap), dtype=BF16)
        xTg[:, :t_e] = xT_full[:, toks[e]].astype(BF16)
        wv = np.zeros((t_cap, 1), dtype=np.float32)
        wv[:t_e, 0] = w_e[e]
        in_maps.append(
            {
                "xT": xTg,
                "dw": down[e].astype(BF16),
                "up": up[e].astype(BF16),
                "wv": wv,
            }
        )

    nc = _build_bass(t_cap)
    res = run_bass_kernel_spmd(nc, in_maps, core_ids=list(range(NCORES)))
    LAST_RESULT = res

    y_all = np.concatenate([res.results[e]["y"] for e in range(E)], axis=0)
    contrib = y_all[pad_pos]  # [N*K, H]
    out = xt + contrib[0::2] + contrib[1::2]
    return out.reshape(B, S, H).astype(np.float32)


# revision 11
# speedup vs baseline: 1.1287x; 1.0881x over previous
"""MoE MLP (top-2 of 8 experts) on 8 Trainium2 NeuronCores.

Strategy: expert parallelism. Each of the 8 cores owns one expert.
Host-side (inside kernel()): route tokens to experts, gather each
expert's tokens into a dense padded [H, T_cap] activation block
(transposed so it is directly usable as the matmul moving operand),
and ship it with that expert's weights to its core. Each core runs
two dense matmuls (down -> relu -> up) entirely out of SBUF and
scales rows by the per-token routing weight. Host-side combine is a
pure gather-add: every token has exactly K=2 expert contributions.

Device compute per core (bf16, fp32 PSUM accumulation):
  hidT[D, T] = down[H, D]^T @ xT[H, T]   (relu)
  y[T, H]    = hidT[D, T]^T @ up[D, H]   (* routing weight per row)
"""

import os
import sys

import numpy as np

for _p in ("/opt/trn_rl_repo", "/root/.axon_site/_ro/trn_rl_repo"):
    if os.path.isdir(_p) and _p not in sys.path:
        sys.path.append(_p)

import ml_dtypes

import concourse.bass as bass
import concourse.mybir as mybir
from concourse import bacc
from concourse.bass_utils import run_bass_kernel_spmd
from concourse.tile import TileContext

BF16 = ml_dtypes.bfloat16

B, S, H, E, K, D = 1, 4096, 1024, 8, 2, 2048
N = B * S
P = 128
KH = H // P   # 8 contraction tiles for the down matmul
KD = D // P   # 16 contraction tiles for the up matmul
NCORES = 8

# Exposed for test harness introspection (exec_time_ns etc).
LAST_RESULT = None


def _build_bass(t_cap: int) -> bass.Bass:
    """One expert's MLP: y[T,H] = w * (relu(x @ down) @ up)."""
    bf16 = mybir.dt.bfloat16
    f32 = mybir.dt.float32
    n_mt = t_cap // P

    nc = bacc.Bacc()
    xT = nc.dram_tensor("xT", [H, t_cap], bf16, kind="ExternalInput")
    dw = nc.dram_tensor("dw", [H, D], bf16, kind="ExternalInput")
    up = nc.dram_tensor("up", [D, H], bf16, kind="ExternalInput")
    wv = nc.dram_tensor("wv", [t_cap, 1], f32, kind="ExternalInput")
    y = nc.dram_tensor("y", [t_cap, H], f32, kind="ExternalOutput")

    with TileContext(nc) as tc:
        with (
            tc.tile_pool(name="const", bufs=1) as const,
            tc.tile_pool(name="psum", bufs=1, space="PSUM") as psum,
            tc.tile_pool(name="outp", bufs=4) as outp,
        ):
            dw_sb = const.tile([P, KH, D], bf16)
            xT_sb = const.tile([P, KH, t_cap], bf16)
            up_sb = const.tile([P, KD, H], bf16)
            hid_sb = const.tile([P, KD, t_cap], bf16)
            wv_sb = const.tile([P, n_mt, 1], f32)

            # Per-chunk loads, interleaved so contraction chunk k of both
            # mm1 operands lands together: the k-outer matmul loop below
            # can start as soon as chunk 0 arrives instead of waiting for
            # the full 6.5MB. dw is further split into column halves so
            # the first half of the D tiles (mh=0 groups) can run while
            # the second half is still in flight. up goes on the same
            # ring strictly after dw so it doesn't steal HBM bandwidth
            # from the critical path; xT streams in parallel on the ACT
            # ring.
            hD = D // 2
            for k in range(KH):
                nc.sync.dma_start(dw_sb[:, k, :hD], dw[k * P : (k + 1) * P, :hD])
                nc.scalar.dma_start(xT_sb[:, k, :], xT[k * P : (k + 1) * P, :])
            for k in range(KH):
                nc.sync.dma_start(dw_sb[:, k, hD:], dw[k * P : (k + 1) * P, hD:])
            nc.gpsimd.dma_start(
                wv_sb[:], wv[:, :].rearrange("(m p) o -> p m o", p=P)
            )
            for k in range(KD):
                nc.sync.dma_start(up_sb[:, k, :], up[k * P : (k + 1) * P, :])

            # mm1: hidT[D, T] = down^T @ xT with relu, k-outermost over 8
            # concurrent PSUM accumulation groups so each matmul only
            # depends on input chunk k.
            chunk = 384 if t_cap % 384 == 0 else 512
            n_off = 0
            while n_off < t_cap:
                n_size = min(chunk, t_cap - n_off)
                for mh in range(KD // 8):
                    pss = [
                        psum.tile([P, n_size], f32, tag=f"ps{m}", name=f"ps{m}")
                        for m in range(8)
                    ]
                    for k in range(KH):
                        for m in range(8):
                            md = mh * 8 + m
                            nc.tensor.matmul(
                                pss[m][:],
                                dw_sb[:, k, md * P : (md + 1) * P],
                                xT_sb[:, k, n_off : n_off + n_size],
                                start=(k == 0),
                                stop=(k == KH - 1),
                            )
                    for m in range(8):
                        md = mh * 8 + m
                        nc.vector.tensor_scalar_max(
                            hid_sb[:, md, n_off : n_off + n_size], pss[m][:], 0.0
                        )
                n_off += n_size

            # mm2: y[T, H] = hidT^T @ up, scaled per token row.
            for mt in range(n_mt):
                for nh in range(H // 512):
                    ps = psum.tile([P, 512], f32, tag=f"ps{(2 * mt + nh) % 8}")
                    for k in range(KD):
                        nc.tensor.matmul(
                            ps[:],
                            hid_sb[:, k, mt * P : (mt + 1) * P],
                            up_sb[:, k, nh * 512 : (nh + 1) * 512],
                            start=(k == 0),
                            stop=(k == KD - 1),
                        )
                    yt = outp.tile([P, 512], f32, tag="yt")
                    nc.vector.tensor_scalar_mul(yt[:], ps[:], wv_sb[:, mt, :])
                    nc.sync.dma_start(
                        y[mt * P : (mt + 1) * P, nh * 512 : (nh + 1) * 512], yt[:]
                    )
    nc.compile()
    return nc


def _route(expert_weights, chosen_expert_indices, attention_mask):
    """Host-side routing. Returns (token ids per expert, weights per
    expert, padded positions per (token, k) pair, T_cap)."""
    idx = np.asarray(chosen_expert_indices).reshape(N, K).astype(np.int64)
    wts = np.asarray(expert_weights).reshape(N, K).astype(np.float32)
    mask = np.asarray(attention_mask).reshape(N, 1).astype(np.float32)
    wts = wts * mask

    flat_e = idx.reshape(-1)  # [N*K]
    order = np.argsort(flat_e, kind="stable")
    counts = np.bincount(flat_e, minlength=E)
    offsets = np.zeros(E + 1, np.int64)
    np.cumsum(counts, out=offsets[1:])
    t_cap = max(P, int(-(-counts.max() // P) * P))

    rank = np.empty(N * K, np.int64)
    rank[order] = np.arange(N * K) - np.repeat(offsets[:-1], counts)
    pad_pos = flat_e * t_cap + rank  # row of pair (n,k) in concat output

    toks = [order[offsets[e] : offsets[e + 1]] // K for e in range(E)]
    w_e = [wts.reshape(-1)[order[offsets[e] : offsets[e + 1]]] for e in range(E)]
    return toks, w_e, pad_pos, t_cap


def kernel(x, attention_mask, expert_weights, chosen_expert_indices, down_proj, up_proj):
    global LAST_RESULT
    xt = np.asarray(x, dtype=np.float32).reshape(N, H)
    toks, w_e, pad_pos, t_cap = _route(
        expert_weights, chosen_expert_indices, attention_mask
    )

    xT_full = np.ascontiguousarray(xt.T)  # [H, N]
    down = np.asarray(down_proj, dtype=np.float32)
    up = np.asarray(up_proj, dtype=np.float32)

    in_maps = []
    for e in range(E):
        t_e = len(toks[e])
        xTg = np.zeros((H, t_cap), dtype=BF16)
        xTg[:, :t_e] = xT_full[:, toks[e]].astype(BF16)
        wv = np.zeros((t_cap, 1), dtype=np.float32)
        wv[:t_e, 0] = w_e[e]
        in_maps.append(
            {
                "xT": xTg,
                "dw": down[e].astype(BF16),
                "up": up[e].astype(BF16),
                "wv": wv,
            }
        )

    nc = _build_bass(t_cap)
    res = run_bass_kernel_spmd(nc, in_maps, core_ids=list(range(NCORES)))
    LAST_RESULT = res

    y_all = np.concatenate([res.results[e]["y"] for e in range(E)], axis=0)
    contrib = y_all[pad_pos]  # [N*K, H]
    out = xt + contrib[0::2] + contrib[1::2]
    return out.reshape(B, S, H).astype(np.float32)


# revision 13
# speedup vs baseline: 1.1308x; 1.0019x over previous
"""MoE MLP (top-2 of 8 experts) on 8 Trainium2 NeuronCores.

Strategy: expert parallelism. Each of the 8 cores owns one expert.
Host-side (inside kernel()): route tokens to experts, gather each
expert's tokens into a dense padded [H, T_cap] activation block
(transposed so it is directly usable as the matmul moving operand),
and ship it with that expert's weights to its core. Each core runs
two dense matmuls (down -> relu -> up) entirely out of SBUF and
scales rows by the per-token routing weight. Host-side combine is a
pure gather-add: every token has exactly K=2 expert contributions.

Device compute per core (bf16, fp32 PSUM accumulation):
  hidT[D, T] = down[H, D]^T @ xT[H, T]   (relu)
  y[T, H]    = hidT[D, T]^T @ up[D, H]   (* routing weight per row)
"""

import os
import sys

import numpy as np

for _p in ("/opt/trn_rl_repo", "/root/.axon_site/_ro/trn_rl_repo"):
    if os.path.isdir(_p) and _p not in sys.path:
        sys.path.append(_p)

import ml_dtypes

import concourse.bass as bass
import concourse.mybir as mybir
from concourse import bacc
from concourse.bass_utils import run_bass_kernel_spmd
from concourse.tile import TileContext

BF16 = ml_dtypes.bfloat16

B, S, H, E, K, D = 1, 4096, 1024, 8, 2, 2048
N = B * S
P = 128
KH = H // P   # 8 contraction tiles for the down matmul
KD = D // P   # 16 contraction tiles for the up matmul
NCORES = 8

# Exposed for test harness introspection (exec_time_ns etc).
LAST_RESULT = None


def _build_bass(t_cap: int) -> bass.Bass:
    """One expert's MLP: y[T,H] = w * (relu(x @ down) @ up)."""
    bf16 = mybir.dt.bfloat16
    f32 = mybir.dt.float32
    n_mt = t_cap // P

    nc = bacc.Bacc()
    xT = nc.dram_tensor("xT", [H, t_cap], bf16, kind="ExternalInput")
    dw = nc.dram_tensor("dw", [H, D], bf16, kind="ExternalInput")
    up = nc.dram_tensor("up", [D, H], bf16, kind="ExternalInput")
    wv = nc.dram_tensor("wv", [t_cap, 1], f32, kind="ExternalInput")
    y = nc.dram_tensor("y", [t_cap, H], f32, kind="ExternalOutput")

    with TileContext(nc) as tc:
        with (
            tc.tile_pool(name="const", bufs=1) as const,
            tc.tile_pool(name="psum", bufs=1, space="PSUM") as psum,
            tc.tile_pool(name="outp", bufs=4) as outp,
        ):
            dw_sb = const.tile([P, KH, D], bf16)
            xT_sb = const.tile([P, KH, t_cap], bf16)
            up_sb = const.tile([P, KD, H], bf16)
            hid_sb = const.tile([P, KD, t_cap], bf16)
            wv_sb = const.tile([P, n_mt, 1], f32)

            # Per-chunk loads, interleaved so contraction chunk k of both
            # mm1 operands lands together: the k-outer matmul loop below
            # can start as soon as chunk 0 arrives instead of waiting for
            # the full 6.5MB. dw is further split into column halves so
            # the first half of the D tiles (mh=0 groups) can run while
            # the second half is still in flight. up goes on the same
            # ring strictly after dw so it doesn't steal HBM bandwidth
            # from the critical path; xT streams in parallel on the ACT
            # ring.
            hD = D // 2
            for k in range(KH):
                nc.sync.dma_start(dw_sb[:, k, :hD], dw[k * P : (k + 1) * P, :hD])
                nc.scalar.dma_start(xT_sb[:, k, :], xT[k * P : (k + 1) * P, :])
            for k in range(KH):
                nc.sync.dma_start(dw_sb[:, k, hD:], dw[k * P : (k + 1) * P, hD:])
            nc.gpsimd.dma_start(
                wv_sb[:], wv[:, :].rearrange("(m p) o -> p m o", p=P)
            )
            for k in range(KD):
                nc.sync.dma_start(up_sb[:, k, :], up[k * P : (k + 1) * P, :])

            # Warm up the PE clock (HAM un-throttles after ~3.4us of
            # sustained activity) with dummy matmuls that depend on
            # nothing but a memset, so the real matmuls below run at
            # 2.4GHz from the start instead of 1.2GHz.
            warm_sb = const.tile([P, 640], bf16)
            nc.vector.memset(warm_sb[:], 0.0)
            warm_ps = psum.tile([P, 512], f32, tag="ps0", name="warm_ps")
            for i in range(16):
                nc.tensor.matmul(
                    warm_ps[:],
                    warm_sb[:, :P],
                    warm_sb[:, P:640],
                    start=(i == 0),
                    stop=(i == 15),
                )

            # mm1: hidT[D, T] = down^T @ xT with relu, k-outermost over 8
            # concurrent PSUM accumulation groups so each matmul only
            # depends on input chunk k.
            chunk = 384 if t_cap % 384 == 0 else 512
            n_off = 0
            while n_off < t_cap:
                n_size = min(chunk, t_cap - n_off)
                for mh in range(KD // 8):
                    pss = [
                        psum.tile([P, n_size], f32, tag=f"ps{m}", name=f"ps{m}")
                        for m in range(8)
                    ]
                    for k in range(KH):
                        for m in range(8):
                            md = mh * 8 + m
                            nc.tensor.matmul(
                                pss[m][:],
                                dw_sb[:, k, md * P : (md + 1) * P],
                                xT_sb[:, k, n_off : n_off + n_size],
                                start=(k == 0),
                                stop=(k == KH - 1),
                            )
                    for m in range(8):
                        md = mh * 8 + m
                        nc.vector.tensor_scalar_max(
                            hid_sb[:, md, n_off : n_off + n_size], pss[m][:], 0.0
                        )
                n_off += n_size

            # mm2: y[T, H] = hidT^T @ up, scaled per token row.
            for mt in range(n_mt):
                for nh in range(H // 512):
                    ps = psum.tile([P, 512], f32, tag=f"ps{(2 * mt + nh) % 8}")
                    for k in range(KD):
                        nc.tensor.matmul(
                            ps[:],
                            hid_sb[:, k, mt * P : (mt + 1) * P],
                            up_sb[:, k, nh * 512 : (nh + 1) * 512],
                            start=(k == 0),
                            stop=(k == KD - 1),
                        )
                    yt = outp.tile([P, 512], f32, tag="yt")
                    nc.vector.tensor_scalar_mul(yt[:], ps[:], wv_sb[:, mt, :])
                    nc.sync.dma_start(
                        y[mt * P : (mt + 1) * P, nh * 512 : (nh + 1) * 512], yt[:]
                    )
    nc.compile()
    return nc


def _route(expert_weights, chosen_expert_indices, attention_mask):
    """Host-side routing. Returns (token ids per expert, weights per
    expert, padded positions per (token, k) pair, T_cap)."""
    idx = np.asarray(chosen_expert_indices).reshape(N, K).astype(np.int64)
    wts = np.asarray(expert_weights).reshape(N, K).astype(np.float32)
    mask = np.asarray(attention_mask).reshape(N, 1).astype(np.float32)
    wts = wts * mask

    flat_e = idx.reshape(-1)  # [N*K]
    order = np.argsort(flat_e, kind="stable")
    counts = np.bincount(flat_e, minlength=E)
    offsets = np.zeros(E + 1, np.int64)
    np.cumsum(counts, out=offsets[1:])
    t_cap = max(P, int(-(-counts.max() // P) * P))

    rank = np.empty(N * K, np.int64)
    rank[order] = np.arange(N * K) - np.repeat(offsets[:-1], counts)
    pad_pos = flat_e * t_cap + rank  # row of pair (n,k) in concat output

    toks = [order[offsets[e] : offsets[e + 1]] // K for e in range(E)]
    w_e = [wts.reshape(-1)[order[offsets[e] : offsets[e + 1]]] for e in range(E)]
    return toks, w_e, pad_pos, t_cap


def kernel(x, attention_mask, expert_weights, chosen_expert_indices, down_proj, up_proj):
    global LAST_RESULT
    xt = np.asarray(x, dtype=np.float32).reshape(N, H)
    toks, w_e, pad_pos, t_cap = _route(
        expert_weights, chosen_expert_indices, attention_mask
    )

    xT_full = np.ascontiguousarray(xt.T)  # [H, N]
    down = np.asarray(down_proj, dtype=np.float32)
    up = np.asarray(up_proj, dtype=np.float32)

    in_maps = []
    for e in range(E):
        t_e = len(toks[e])
        xTg = np.zeros((H, t_cap), dtype=BF16)
        xTg[:, :t_e] = xT_full[:, toks[e]].astype(BF16)
        wv = np.zeros((t_cap, 1), dtype=np.float32)
        wv[:t_e, 0] = w_e[e]
        in_maps.append(
            {
                "xT": xTg,
                "dw": down[e].astype(BF16),
                "up": up[e].astype(BF16),
                "wv": wv,
            }
        )

    nc = _build_bass(t_cap)
    res = run_bass_kernel_spmd(nc, in_maps, core_ids=list(range(NCORES)))
    LAST_RESULT = res

    y_all = np.concatenate([res.results[e]["y"] for e in range(E)], axis=0)
    contrib = y_all[pad_pos]  # [N*K, H]
    out = xt + contrib[0::2] + contrib[1::2]
    return out.reshape(B, S, H).astype(np.float32)


# revision 15
# speedup vs baseline: 1.1526x; 1.0193x over previous
"""MoE MLP (top-2 of 8 experts) on 8 Trainium2 NeuronCores.

Strategy: expert parallelism. Each of the 8 cores owns one expert.
Host-side (inside kernel()): route tokens to experts, gather each
expert's tokens into a dense padded [H, T_cap] activation block
(transposed so it is directly usable as the matmul moving operand),
and ship it with that expert's weights to its core. Each core runs
two dense matmuls (down -> relu -> up) entirely out of SBUF and
scales rows by the per-token routing weight. Host-side combine is a
pure gather-add: every token has exactly K=2 expert contributions.

Device compute per core (bf16, fp32 PSUM accumulation):
  hidT[D, T] = down[H, D]^T @ xT[H, T]   (relu)
  y[T, H]    = hidT[D, T]^T @ up[D, H]   (* routing weight per row)
"""

import os
import sys

import numpy as np

for _p in ("/opt/trn_rl_repo", "/root/.axon_site/_ro/trn_rl_repo"):
    if os.path.isdir(_p) and _p not in sys.path:
        sys.path.append(_p)

import ml_dtypes

import concourse.bass as bass
import concourse.mybir as mybir
from concourse import bacc
from concourse.bass_utils import run_bass_kernel_spmd
from concourse.tile import TileContext

BF16 = ml_dtypes.bfloat16

B, S, H, E, K, D = 1, 4096, 1024, 8, 2, 2048
N = B * S
P = 128
KH = H // P   # 8 contraction tiles for the down matmul
KD = D // P   # 16 contraction tiles for the up matmul
NCORES = 8

# Exposed for test harness introspection (exec_time_ns etc).
LAST_RESULT = None


def _build_bass(t_cap: int) -> bass.Bass:
    """One expert's MLP: y[T,H] = w * (relu(x @ down) @ up)."""
    bf16 = mybir.dt.bfloat16
    f32 = mybir.dt.float32
    n_mt = t_cap // P

    nc = bacc.Bacc()
    xT = nc.dram_tensor("xT", [H, t_cap], bf16, kind="ExternalInput")
    dw = nc.dram_tensor("dw", [H, D], bf16, kind="ExternalInput")
    up = nc.dram_tensor("up", [D, H], bf16, kind="ExternalInput")
    wv = nc.dram_tensor("wv", [t_cap, 1], f32, kind="ExternalInput")
    y = nc.dram_tensor("y", [t_cap, H], f32, kind="ExternalOutput")

    with TileContext(nc) as tc:
        with (
            tc.tile_pool(name="const", bufs=1) as const,
            tc.tile_pool(name="psum", bufs=1, space="PSUM") as psum,
            tc.tile_pool(name="outp", bufs=4) as outp,
        ):
            dw_sb = const.tile([P, KH, D], bf16)
            xT_sb = const.tile([P, KH, t_cap], bf16)
            up_sb = const.tile([P, KD, H], bf16)
            hid_sb = const.tile([P, KD, t_cap], bf16)
            wv_sb = const.tile([P, n_mt, 1], f32)

            # Per-chunk loads, interleaved so contraction chunk k of both
            # mm1 operands lands together: the k-outer matmul loop below
            # can start as soon as chunk 0 arrives instead of waiting for
            # the full 6.5MB. dw is further split into column halves so
            # the first half of the D tiles (mh=0 groups) can run while
            # the second half is still in flight. up goes on the same
            # ring strictly after dw so it doesn't steal HBM bandwidth
            # from the critical path; xT streams in parallel on the ACT
            # ring.
            hD = D // 2
            chunk = 384 if t_cap % 384 == 0 else 512
            n0 = min(chunk, t_cap)
            rings = [nc.sync, nc.scalar]
            # Phase 1 (critical path): weight half A + the first token
            # chunk, alternating rings per k so chunk k of both operands
            # lands at PE consumption rate.
            for k in range(KH):
                rings[k % 2].dma_start(
                    dw_sb[:, k, :hD], dw[k * P : (k + 1) * P, :hD]
                )
                rings[1 - k % 2].dma_start(
                    xT_sb[:, k, :n0], xT[k * P : (k + 1) * P, :n0]
                )
            # Phase 2: weight half B (needed from ~21us).
            for k in range(KH):
                rings[k % 2].dma_start(
                    dw_sb[:, k, hD:], dw[k * P : (k + 1) * P, hD:]
                )
            # Phase 3: remaining token chunks.
            off = n0
            while off < t_cap:
                sz = min(chunk, t_cap - off)
                for k in range(KH):
                    rings[k % 2].dma_start(
                        xT_sb[:, k, off : off + sz],
                        xT[k * P : (k + 1) * P, off : off + sz],
                    )
                off += sz
            nc.gpsimd.dma_start(
                wv_sb[:], wv[:, :].rearrange("(m p) o -> p m o", p=P)
            )
            # Phase 4: up weights (needed only when mm2 starts ~75us).
            for k in range(KD):
                rings[k % 2].dma_start(up_sb[:, k, :], up[k * P : (k + 1) * P, :])

            # Warm up the PE clock (HAM un-throttles after ~3.4us of
            # sustained activity) with dummy matmuls that depend on
            # nothing but a memset, so the real matmuls below run at
            # 2.4GHz from the start instead of 1.2GHz.
            warm_sb = const.tile([P, 640], bf16)
            nc.vector.memset(warm_sb[:], 0.0)
            warm_ps = psum.tile([P, 512], f32, tag="ps0", name="warm_ps")
            for i in range(12):
                nc.tensor.matmul(
                    warm_ps[:],
                    warm_sb[:, :P],
                    warm_sb[:, P:640],
                    start=(i == 0),
                    stop=(i == 11),
                )

            # mm1: hidT[D, T] = down^T @ xT with relu, k-outermost over 8
            # concurrent PSUM accumulation groups so each matmul only
            # depends on input chunk k.
            chunk = 384 if t_cap % 384 == 0 else 512
            n_off = 0
            while n_off < t_cap:
                n_size = min(chunk, t_cap - n_off)
                for mh in range(KD // 8):
                    pss = [
                        psum.tile([P, n_size], f32, tag=f"ps{m}", name=f"ps{m}")
                        for m in range(8)
                    ]
                    for k in range(KH):
                        for m in range(8):
                            md = mh * 8 + m
                            nc.tensor.matmul(
                                pss[m][:],
                                dw_sb[:, k, md * P : (md + 1) * P],
                                xT_sb[:, k, n_off : n_off + n_size],
                                start=(k == 0),
                                stop=(k == KH - 1),
                            )
                    for m in range(8):
                        md = mh * 8 + m
                        nc.vector.tensor_scalar_max(
                            hid_sb[:, md, n_off : n_off + n_size], pss[m][:], 0.0
                        )
                n_off += n_size

            # mm2: y[T, H] = hidT^T @ up, scaled per token row.
            for mt in range(n_mt):
                for nh in range(H // 512):
                    ps = psum.tile([P, 512], f32, tag=f"ps{(2 * mt + nh) % 8}")
                    for k in range(KD):
                        nc.tensor.matmul(
                            ps[:],
                            hid_sb[:, k, mt * P : (mt + 1) * P],
                            up_sb[:, k, nh * 512 : (nh + 1) * 512],
                            start=(k == 0),
                            stop=(k == KD - 1),
                        )
                    yt = outp.tile([P, 512], f32, tag="yt")
                    nc.vector.tensor_scalar_mul(yt[:], ps[:], wv_sb[:, mt, :])
                    nc.sync.dma_start(
                        y[mt * P : (mt + 1) * P, nh * 512 : (nh + 1) * 512], yt[:]
                    )
    nc.compile()
    return nc


def _route(expert_weights, chosen_expert_indices, attention_mask):
    """Host-side routing. Returns (token ids per expert, weights per
    expert, padded positions per (token, k) pair, T_cap)."""
    idx = np.asarray(chosen_expert_indices).reshape(N, K).astype(np.int64)
    wts = np.asarray(expert_weights).reshape(N, K).astype(np.float32)
    mask = np.asarray(attention_mask).reshape(N, 1).astype(np.float32)
    wts = wts * mask

    flat_e = idx.reshape(-1)  # [N*K]
    order = np.argsort(flat_e, kind="stable")
    counts = np.bincount(flat_e, minlength=E)
    offsets = np.zeros(E + 1, np.int64)
    np.cumsum(counts, out=offsets[1:])
    t_cap = max(P, int(-(-counts.max() // P) * P))

    rank = np.empty(N * K, np.int64)
    rank[order] = np.arange(N * K) - np.repeat(offsets[:-1], counts)
    pad_pos = flat_e * t_cap + rank  # row of pair (n,k) in concat output

    toks = [order[offsets[e] : offsets[e + 1]] // K for e in range(E)]
    w_e = [wts.reshape(-1)[order[offsets[e] : offsets[e + 1]]] for e in range(E)]
    return toks, w_e, pad_pos, t_cap


def kernel(x, attention_mask, expert_weights, chosen_expert_indices, down_proj, up_proj):
    global LAST_RESULT
    xt = np.asarray(x, dtype=np.float32).reshape(N, H)
    toks, w_e, pad_pos, t_cap = _route(
        expert_weights, chosen_expert_indices, attention_mask
    )

    xT_full = np.ascontiguousarray(xt.T)  # [H, N]
    down = np.asarray(down_proj, dtype=np.float32)
    up = np.asarray(up_proj, dtype=np.float32)

    in_maps = []
    for e in range(E):
        t_e = len(toks[e])
        xTg = np.zeros((H, t_cap), dtype=BF16)
        xTg[:, :t_e] = xT_full[:, toks[e]].astype(BF16)
        wv = np.zeros((t_cap, 1), dtype=np.float32)
        wv[:t_e, 0] = w_e[e]
        in_maps.append(
            {
                "xT": xTg,
                "dw": down[e].astype(BF16),
                "up": up[e].astype(BF16),
                "wv": wv,
            }
        )

    nc = _build_bass(t_cap)
    res = run_bass_kernel_spmd(nc, in_maps, core_ids=list(range(NCORES)))
    LAST_RESULT = res

    y_all = np.concatenate([res.results[e]["y"] for e in range(E)], axis=0)
    contrib = y_all[pad_pos]  # [N*K, H]
    out = xt + contrib[0::2] + contrib[1::2]
    return out.reshape(B, S, H).astype(np.float32)


# revision 22
# speedup vs baseline: 1.1570x; 1.0038x over previous
"""MoE MLP (top-2 of 8 experts) on 8 Trainium2 NeuronCores.

Strategy: expert parallelism. Each of the 8 cores owns one expert.
Host-side (inside kernel()): route tokens to experts, gather each
expert's tokens into a dense padded [H, T_cap] activation block
(transposed so it is directly usable as the matmul moving operand),
and ship it with that expert's weights to its core. Each core runs
two dense matmuls (down -> relu -> up) entirely out of SBUF and
scales rows by the per-token routing weight. Host-side combine is a
pure gather-add: every token has exactly K=2 expert contributions.

Device compute per core (bf16, fp32 PSUM accumulation):
  hidT[D, T] = down[H, D]^T @ xT[H, T]   (relu)
  y[T, H]    = hidT[D, T]^T @ up[D, H]   (* routing weight per row)
"""

import os
import sys

import numpy as np

for _p in ("/opt/trn_rl_repo", "/root/.axon_site/_ro/trn_rl_repo"):
    if os.path.isdir(_p) and _p not in sys.path:
        sys.path.append(_p)

import ml_dtypes

import concourse.bass as bass
import concourse.mybir as mybir
from concourse import bacc
from concourse.bass_utils import run_bass_kernel_spmd
from concourse.tile import TileContext

BF16 = ml_dtypes.bfloat16

B, S, H, E, K, D = 1, 4096, 1024, 8, 2, 2048
N = B * S
P = 128
KH = H // P   # 8 contraction tiles for the down matmul
KD = D // P   # 16 contraction tiles for the up matmul
NCORES = 8

# Exposed for test harness introspection (exec_time_ns etc).
LAST_RESULT = None


def _chunks(total: int, maxc: int = 512) -> list[tuple[int, int]]:
    """Equal-ish (offset, size) split of `total` into ceil(total/maxc)
    pieces — keeps every matmul moving-dim well above the dispatch
    floor instead of leaving a tiny remainder chunk."""
    n = -(-total // maxc)
    base, rem = divmod(total, n)
    out, off = [], 0
    for i in range(n):
        sz = base + (1 if i < rem else 0)
        out.append((off, sz))
        off += sz
    return out


def _build_bass(t_cap: int) -> bass.Bass:
    """One expert's MLP: y[T,H] = w * (relu(x @ down) @ up)."""
    bf16 = mybir.dt.bfloat16
    f32 = mybir.dt.float32
    n_mt = -(-t_cap // P)  # last token tile may be partial

    nc = bacc.Bacc()
    xT = nc.dram_tensor("xT", [H, t_cap], bf16, kind="ExternalInput")
    dw = nc.dram_tensor("dw", [H, D], bf16, kind="ExternalInput")
    up = nc.dram_tensor("up", [D, H], bf16, kind="ExternalInput")
    wv = nc.dram_tensor("wv", [n_mt * P, 1], f32, kind="ExternalInput")
    y = nc.dram_tensor("y", [t_cap, H], f32, kind="ExternalOutput")

    with TileContext(nc) as tc:
        with (
            tc.tile_pool(name="const", bufs=1) as const,
            tc.tile_pool(name="psum", bufs=1, space="PSUM") as psum,
            tc.tile_pool(name="outp", bufs=4) as outp,
        ):
            dw_sb = const.tile([P, KH, D], bf16)
            xT_sb = const.tile([P, KH, t_cap], bf16)
            up_sb = const.tile([P, KD, H], bf16)
            hid_sb = const.tile([P, KD, t_cap], bf16)
            wv_sb = const.tile([P, n_mt, 1], f32)

            # Per-chunk loads, interleaved so contraction chunk k of both
            # mm1 operands lands together: the k-outer matmul loop below
            # can start as soon as chunk 0 arrives instead of waiting for
            # the full 6.5MB. dw is further split into column halves so
            # the first half of the D tiles (mh=0 groups) can run while
            # the second half is still in flight. up goes on the same
            # ring strictly after dw so it doesn't steal HBM bandwidth
            # from the critical path; xT streams in parallel on the ACT
            # ring.
            hD = D // 2
            tchunks = _chunks(t_cap)
            n0_off, n0 = tchunks[0]
            rings = [nc.sync, nc.scalar]
            # Phase 1 (critical path): weight half A + the first token
            # chunk, alternating rings per k so chunk k of both operands
            # lands at PE consumption rate.
            for k in range(KH):
                rings[k % 2].dma_start(
                    dw_sb[:, k, :hD], dw[k * P : (k + 1) * P, :hD]
                )
                rings[1 - k % 2].dma_start(
                    xT_sb[:, k, :n0], xT[k * P : (k + 1) * P, :n0]
                )
            # Phase 2: weight half B (needed from ~21us).
            for k in range(KH):
                rings[k % 2].dma_start(
                    dw_sb[:, k, hD:], dw[k * P : (k + 1) * P, hD:]
                )
            # Phase 3: remaining token chunks.
            for off, sz in tchunks[1:]:
                for k in range(KH):
                    rings[k % 2].dma_start(
                        xT_sb[:, k, off : off + sz],
                        xT[k * P : (k + 1) * P, off : off + sz],
                    )
            nc.gpsimd.dma_start(
                wv_sb[:], wv[:, :].rearrange("(m p) o -> p m o", p=P)
            )
            # Phase 4: up weights (needed only when mm2 starts ~75us).
            for k in range(KD):
                rings[k % 2].dma_start(up_sb[:, k, :], up[k * P : (k + 1) * P, :])

            # Warm up the PE clock (HAM un-throttles after ~3.4us of
            # sustained activity) with dummy matmuls that depend on
            # nothing but a memset, so the real matmuls below run at
            # 2.4GHz from the start instead of 1.2GHz.
            warm_sb = const.tile([P, 640], bf16)
            nc.vector.memset(warm_sb[:], 0.0)
            warm_ps = psum.tile([P, 512], f32, tag="ps0", name="warm_ps")
            for i in range(12):
                nc.tensor.matmul(
                    warm_ps[:],
                    warm_sb[:, :P],
                    warm_sb[:, P:640],
                    start=(i == 0),
                    stop=(i == 11),
                )

            # mm1: hidT[D, T] = down^T @ xT with relu, k-outermost over 8
            # concurrent PSUM accumulation groups so each matmul only
            # depends on input chunk k.
            for n_off, n_size in tchunks:
                for mh in range(KD // 8):
                    pss = [
                        psum.tile([P, n_size], f32, tag=f"ps{m}", name=f"ps{m}")
                        for m in range(8)
                    ]
                    for k in range(KH):
                        for m in range(8):
                            md = mh * 8 + m
                            nc.tensor.matmul(
                                pss[m][:],
                                dw_sb[:, k, md * P : (md + 1) * P],
                                xT_sb[:, k, n_off : n_off + n_size],
                                start=(k == 0),
                                stop=(k == KH - 1),
                            )
                    for m in range(8):
                        md = mh * 8 + m
                        nc.vector.tensor_scalar_max(
                            hid_sb[:, md, n_off : n_off + n_size], pss[m][:], 0.0
                        )

            # mm2: y[T, H] = hidT^T @ up, scaled per token row. The last
            # token tile may have fewer than 128 rows.
            for mt in range(n_mt):
                mp = min(P, t_cap - mt * P)
                for nh in range(H // 512):
                    ps = psum.tile([mp, 512], f32, tag=f"ps{(2 * mt + nh) % 8}")
                    for k in range(KD):
                        nc.tensor.matmul(
                            ps[:],
                            hid_sb[:, k, mt * P : mt * P + mp],
                            up_sb[:, k, nh * 512 : (nh + 1) * 512],
                            start=(k == 0),
                            stop=(k == KD - 1),
                        )
                    yt = outp.tile([mp, 512], f32, tag="yt")
                    nc.vector.tensor_scalar_mul(yt[:], ps[:], wv_sb[:mp, mt, :])
                    nc.sync.dma_start(
                        y[mt * P : mt * P + mp, nh * 512 : (nh + 1) * 512], yt[:]
                    )
    nc.compile()
    return nc


def _route(expert_weights, chosen_expert_indices, attention_mask):
    """Host-side routing. Returns (token ids per expert, weights per
    expert, padded positions per (token, k) pair, T_cap)."""
    idx = np.asarray(chosen_expert_indices).reshape(N, K).astype(np.int64)
    wts = np.asarray(expert_weights).reshape(N, K).astype(np.float32)
    mask = np.asarray(attention_mask).reshape(N, 1).astype(np.float32)
    wts = wts * mask

    flat_e = idx.reshape(-1)  # [N*K]
    order = np.argsort(flat_e, kind="stable")
    counts = np.bincount(flat_e, minlength=E)
    offsets = np.zeros(E + 1, np.int64)
    np.cumsum(counts, out=offsets[1:])
    t_cap = max(P, int(counts.max()))

    rank = np.empty(N * K, np.int64)
    rank[order] = np.arange(N * K) - np.repeat(offsets[:-1], counts)
    pad_pos = flat_e * t_cap + rank  # row of pair (n,k) in concat output

    toks = [order[offsets[e] : offsets[e + 1]] // K for e in range(E)]
    w_e = [wts.reshape(-1)[order[offsets[e] : offsets[e + 1]]] for e in range(E)]
    return toks, w_e, pad_pos, t_cap


def kernel(x, attention_mask, expert_weights, chosen_expert_indices, down_proj, up_proj):
    global LAST_RESULT
    xt = np.asarray(x, dtype=np.float32).reshape(N, H)
    toks, w_e, pad_pos, t_cap = _route(
        expert_weights, chosen_expert_indices, attention_mask
    )

    xT_full = np.ascontiguousarray(xt.T)  # [H, N]
    down = np.asarray(down_proj, dtype=np.float32)
    up = np.asarray(up_proj, dtype=np.float32)

    n_mt = -(-t_cap // P)
    in_maps = []
    for e in range(E):
        t_e = len(toks[e])
        xTg = np.zeros((H, t_cap), dtype=BF16)
        xTg[:, :t_e] = xT_full[:, toks[e]].astype(BF16)
        wv = np.zeros((n_mt * P, 1), dtype=np.float32)
        wv[:t_e, 0] = w_e[e]
        in_maps.append(
            {
                "xT": xTg,
                "dw": down[e].astype(BF16),
                "up": up[e].astype(BF16),
                "wv": wv,
            }
        )

    nc = _build_bass(t_cap)
    res = run_bass_kernel_spmd(nc, in_maps, core_ids=list(range(NCORES)))
    LAST_RESULT = res

    y_all = np.concatenate([res.results[e]["y"] for e in range(E)], axis=0)
    contrib = y_all[pad_pos]  # [N*K, H]
    out = xt + contrib[0::2] + contrib[1::2]
    return out.reshape(B, S, H).astype(np.float32)


# revision 26
# speedup vs baseline: 1.1820x; 1.0215x over previous
"""MoE MLP (top-2 of 8 experts) on 8 Trainium2 NeuronCores.

Strategy: expert parallelism. Each of the 8 cores owns one expert.
Host-side (inside kernel()): route tokens to experts, gather each
expert's tokens into a dense padded [H, T_cap] activation block
(transposed so it is directly usable as the matmul moving operand),
and ship it with that expert's weights to its core. Each core runs
two dense matmuls (down -> relu -> up) entirely out of SBUF and
scales rows by the per-token routing weight. Host-side combine is a
pure gather-add: every token has exactly K=2 expert contributions.

Device compute per core (bf16, fp32 PSUM accumulation):
  hidT[D, T] = down[H, D]^T @ xT[H, T]   (relu)
  y[T, H]    = hidT[D, T]^T @ up[D, H]   (* routing weight per row)
"""

import os
import sys

import numpy as np

for _p in ("/opt/trn_rl_repo", "/root/.axon_site/_ro/trn_rl_repo"):
    if os.path.isdir(_p) and _p not in sys.path:
        sys.path.append(_p)

import ml_dtypes

import concourse.bass as bass
import concourse.mybir as mybir
from concourse import bacc
from concourse.bass_utils import run_bass_kernel_spmd
from concourse.tile import TileContext

BF16 = ml_dtypes.bfloat16

B, S, H, E, K, D = 1, 4096, 1024, 8, 2, 2048
N = B * S
P = 128
KH = H // P   # 8 contraction tiles for the down matmul
KD = D // P   # 16 contraction tiles for the up matmul
NCORES = 8

# Exposed for test harness introspection (exec_time_ns etc).
LAST_RESULT = None


def _chunks(total: int, maxc: int = 512) -> list[tuple[int, int]]:
    """Equal-ish (offset, size) split of `total` into ceil(total/maxc)
    pieces — keeps every matmul moving-dim well above the dispatch
    floor instead of leaving a tiny remainder chunk."""
    n = -(-total // maxc)
    base, rem = divmod(total, n)
    out, off = [], 0
    for i in range(n):
        sz = base + (1 if i < rem else 0)
        out.append((off, sz))
        off += sz
    return out


def _build_bass(t_cap: int) -> bass.Bass:
    """One expert's MLP: y[T,H] = w * (relu(x @ down) @ up)."""
    bf16 = mybir.dt.bfloat16
    f32 = mybir.dt.float32
    n_mt = -(-t_cap // P)  # last token tile may be partial

    t_round = n_mt * P

    nc = bacc.Bacc()
    xT = nc.dram_tensor("xT", [H, t_cap], bf16, kind="ExternalInput")
    dw = nc.dram_tensor("dw", [H, D], bf16, kind="ExternalInput")
    up = nc.dram_tensor("up", [D, H], bf16, kind="ExternalInput")
    wv = nc.dram_tensor("wv", [t_round, 1], f32, kind="ExternalInput")
    y = nc.dram_tensor("y", [t_round, H], f32, kind="ExternalOutput")

    with TileContext(nc) as tc:
        with (
            tc.tile_pool(name="const", bufs=1) as const,
            tc.tile_pool(name="psum", bufs=1, space="PSUM") as psum,
            tc.tile_pool(name="outp", bufs=4) as outp,
        ):
            dw_sb = const.tile([P, KH, D], bf16)
            xT_sb = const.tile([P, KH, t_cap], bf16)
            up_sb = const.tile([P, KD, H], bf16)
            hid_sb = const.tile([P, KD, t_round], bf16)
            wv_sb = const.tile([P, n_mt, 1], f32)
            if t_round > t_cap:
                # mm1 only fills hid up to t_cap; zero the padded token
                # columns so mm2 can use full 128-row tiles throughout
                # (partial-partition output stores drain serially and
                # cost ~5us on the critical tail).
                nc.vector.memset(hid_sb[:, :, t_cap:], 0.0)

            # Per-chunk loads, interleaved so contraction chunk k of both
            # mm1 operands lands together: the k-outer matmul loop below
            # can start as soon as chunk 0 arrives instead of waiting for
            # the full 6.5MB. dw is further split into column halves so
            # the first half of the D tiles (mh=0 groups) can run while
            # the second half is still in flight. up goes on the same
            # ring strictly after dw so it doesn't steal HBM bandwidth
            # from the critical path; xT streams in parallel on the ACT
            # ring.
            hD = D // 2
            tchunks = _chunks(t_cap)
            n0_off, n0 = tchunks[0]
            rings = [nc.sync, nc.scalar]
            # Phase 1 (critical path): weight half A + the first token
            # chunk, alternating rings per k so chunk k of both operands
            # lands at PE consumption rate.
            for k in range(KH):
                rings[k % 2].dma_start(
                    dw_sb[:, k, :hD], dw[k * P : (k + 1) * P, :hD]
                )
                rings[1 - k % 2].dma_start(
                    xT_sb[:, k, :n0], xT[k * P : (k + 1) * P, :n0]
                )
            # Phase 2: weight half B (needed from ~21us).
            for k in range(KH):
                rings[k % 2].dma_start(
                    dw_sb[:, k, hD:], dw[k * P : (k + 1) * P, hD:]
                )
            # Phase 3: remaining token chunks.
            for off, sz in tchunks[1:]:
                for k in range(KH):
                    rings[k % 2].dma_start(
                        xT_sb[:, k, off : off + sz],
                        xT[k * P : (k + 1) * P, off : off + sz],
                    )
            nc.gpsimd.dma_start(
                wv_sb[:], wv[:, :].rearrange("(m p) o -> p m o", p=P)
            )
            # Phase 4: up weights (needed only when mm2 starts ~75us).
            for k in range(KD):
                rings[k % 2].dma_start(up_sb[:, k, :], up[k * P : (k + 1) * P, :])

            # Warm up the PE clock (HAM un-throttles after ~3.4us of
            # sustained activity) with dummy matmuls that depend on
            # nothing but a memset, so the real matmuls below run at
            # 2.4GHz from the start instead of 1.2GHz.
            warm_sb = const.tile([P, 640], bf16)
            nc.vector.memset(warm_sb[:], 0.0)
            warm_ps = psum.tile([P, 512], f32, tag="ps0", name="warm_ps")
            for i in range(12):
                nc.tensor.matmul(
                    warm_ps[:],
                    warm_sb[:, :P],
                    warm_sb[:, P:640],
                    start=(i == 0),
                    stop=(i == 11),
                )

            # mm1: hidT[D, T] = down^T @ xT with relu, k-outermost over 8
            # concurrent PSUM accumulation groups so each matmul only
            # depends on input chunk k.
            for n_off, n_size in tchunks:
                for mh in range(KD // 8):
                    pss = [
                        psum.tile([P, n_size], f32, tag=f"ps{m}", name=f"ps{m}")
                        for m in range(8)
                    ]
                    for k in range(KH):
                        for m in range(8):
                            md = mh * 8 + m
                            nc.tensor.matmul(
                                pss[m][:],
                                dw_sb[:, k, md * P : (md + 1) * P],
                                xT_sb[:, k, n_off : n_off + n_size],
                                start=(k == 0),
                                stop=(k == KH - 1),
                            )
                    for m in range(8):
                        md = mh * 8 + m
                        nc.vector.tensor_scalar_max(
                            hid_sb[:, md, n_off : n_off + n_size], pss[m][:], 0.0
                        )

            # mm2: y[T, H] = hidT^T @ up, scaled per token row.
            for mt in range(n_mt):
                for nh in range(H // 512):
                    ps = psum.tile([P, 512], f32, tag=f"ps{(2 * mt + nh) % 8}")
                    for k in range(KD):
                        nc.tensor.matmul(
                            ps[:],
                            hid_sb[:, k, mt * P : (mt + 1) * P],
                            up_sb[:, k, nh * 512 : (nh + 1) * 512],
                            start=(k == 0),
                            stop=(k == KD - 1),
                        )
                    yt = outp.tile([P, 512], f32, tag="yt")
                    nc.vector.tensor_scalar_mul(yt[:], ps[:], wv_sb[:, mt, :])
                    nc.sync.dma_start(
                        y[mt * P : (mt + 1) * P, nh * 512 : (nh + 1) * 512], yt[:]
                    )
    nc.compile()
    return nc


def _route(expert_weights, chosen_expert_indices, attention_mask):
    """Host-side routing. Returns (token ids per expert, weights per
    expert, padded positions per (token, k) pair, T_cap)."""
    idx = np.asarray(chosen_expert_indices).reshape(N, K).astype(np.int64)
    wts = np.asarray(expert_weights).reshape(N, K).astype(np.float32)
    mask = np.asarray(attention_mask).reshape(N, 1).astype(np.float32)
    wts = wts * mask

    flat_e = idx.reshape(-1)  # [N*K]
    order = np.argsort(flat_e, kind="stable")
    counts = np.bincount(flat_e, minlength=E)
    offsets = np.zeros(E + 1, np.int64)
    np.cumsum(counts, out=offsets[1:])
    t_cap = max(P, int(counts.max()))

    t_round = -(-t_cap // P) * P  # device output rows per expert
    rank = np.empty(N * K, np.int64)
    rank[order] = np.arange(N * K) - np.repeat(offsets[:-1], counts)
    pad_pos = flat_e * t_round + rank  # row of pair (n,k) in concat output

    toks = [order[offsets[e] : offsets[e + 1]] // K for e in range(E)]
    w_e = [wts.reshape(-1)[order[offsets[e] : offsets[e + 1]]] for e in range(E)]
    return toks, w_e, pad_pos, t_cap


def kernel(x, attention_mask, expert_weights, chosen_expert_indices, down_proj, up_proj):
    global LAST_RESULT
    xt = np.asarray(x, dtype=np.float32).reshape(N, H)
    toks, w_e, pad_pos, t_cap = _route(
        expert_weights, chosen_expert_indices, attention_mask
    )

    xT_full = np.ascontiguousarray(xt.T)  # [H, N]
    down = np.asarray(down_proj, dtype=np.float32)
    up = np.asarray(up_proj, dtype=np.float32)

    n_mt = -(-t_cap // P)
    in_maps = []
    for e in range(E):
        t_e = len(toks[e])
        xTg = np.zeros((H, t_cap), dtype=BF16)
        xTg[:, :t_e] = xT_full[:, toks[e]].astype(BF16)
        wv = np.zeros((n_mt * P, 1), dtype=np.float32)
        wv[:t_e, 0] = w_e[e]
        in_maps.append(
            {
                "xT": xTg,
                "dw": down[e].astype(BF16),
                "up": up[e].astype(BF16),
                "wv": wv,
            }
        )

    nc = _build_bass(t_cap)
    res = run_bass_kernel_spmd(nc, in_maps, core_ids=list(range(NCORES)))
    LAST_RESULT = res

    y_all = np.concatenate([res.results[e]["y"] for e in range(E)], axis=0)
    contrib = y_all[pad_pos]  # [N*K, H]
    out = xt + contrib[0::2] + contrib[1::2]
    return out.reshape(B, S, H).astype(np.float32)


# revision 33
# speedup vs baseline: 1.2273x; 1.0383x over previous
"""MoE MLP (top-2 of 8 experts) on 8 Trainium2 NeuronCores.

Strategy: expert parallelism. Each of the 8 cores owns one expert.
Host-side (inside kernel()): route tokens to experts, gather each
expert's tokens into a dense padded [H, T_cap] activation block
(transposed so it is directly usable as the matmul moving operand),
and ship it with that expert's weights to its core. Each core runs
two dense matmuls (down -> relu -> up) entirely out of SBUF and
scales rows by the per-token routing weight. Host-side combine is a
pure gather-add: every token has exactly K=2 expert contributions.

Device compute per core (bf16, fp32 PSUM accumulation):
  hidT[D, T] = down[H, D]^T @ (w * xT)[H, T]   (relu)
  yT[H, T]   = up[D, H]^T @ hidT[D, T]

The routing weight is folded into the gathered activations on the
host: w >= 0 (uniform routing weights times a 0/1 attention mask) and
relu is positively homogeneous, so relu((w*x) @ down) @ up =
w * (relu(x @ down) @ up). That removes the on-device scale pass and
lets both matmuls stream exactly T real token columns with all-full
128-row output tiles.
"""

import os
import sys

import numpy as np

for _p in ("/opt/trn_rl_repo", "/root/.axon_site/_ro/trn_rl_repo"):
    if os.path.isdir(_p) and _p not in sys.path:
        sys.path.append(_p)

import ml_dtypes

import concourse.bass as bass
import concourse.mybir as mybir
from concourse import bacc
from concourse.bass_utils import run_bass_kernel_spmd
from concourse.tile import TileContext

BF16 = ml_dtypes.bfloat16

B, S, H, E, K, D = 1, 4096, 1024, 8, 2, 2048
N = B * S
P = 128
KH = H // P   # 8 contraction tiles for the down matmul
KD = D // P   # 16 contraction tiles for the up matmul
NCORES = 8

# Exposed for test harness introspection (exec_time_ns etc).
LAST_RESULT = None


def _chunks(total: int, maxc: int = 512) -> list[tuple[int, int]]:
    """Equal-ish (offset, size) split of `total` into ceil(total/maxc)
    pieces — keeps every matmul moving-dim well above the dispatch
    floor instead of leaving a tiny remainder chunk."""
    n = -(-total // maxc)
    base, rem = divmod(total, n)
    out, off = [], 0
    for i in range(n):
        sz = base + (1 if i < rem else 0)
        out.append((off, sz))
        off += sz
    return out


def _build_bass(t_cap: int) -> bass.Bass:
    """One expert's MLP: yT[H,T] = up^T @ relu(down^T @ xT)."""
    bf16 = mybir.dt.bfloat16
    f32 = mybir.dt.float32

    nc = bacc.Bacc()
    xT = nc.dram_tensor("xT", [H, t_cap], bf16, kind="ExternalInput")
    dw = nc.dram_tensor("dw", [H, D], bf16, kind="ExternalInput")
    up = nc.dram_tensor("up", [D, H], bf16, kind="ExternalInput")
    yT = nc.dram_tensor("yT", [H, t_cap], f32, kind="ExternalOutput")

    with TileContext(nc) as tc:
        with (
            tc.tile_pool(name="const", bufs=1) as const,
            tc.tile_pool(name="psum", bufs=1, space="PSUM") as psum,
            tc.tile_pool(name="outp", bufs=4) as outp,
        ):
            dw_sb = const.tile([P, KH, D], bf16)
            xT_sb = const.tile([P, KH, t_cap], bf16)
            up_sb = const.tile([P, KD, H], bf16)
            hid_sb = const.tile([P, KD, t_cap], bf16)

            # Per-chunk loads, interleaved so contraction chunk k of both
            # mm1 operands lands together: the k-outer matmul loop below
            # can start as soon as chunk 0 arrives instead of waiting for
            # the full 6.5MB. dw is further split into column halves so
            # the first half of the D tiles (mh=0 groups) can run while
            # the second half is still in flight. up goes on the same
            # ring strictly after dw so it doesn't steal HBM bandwidth
            # from the critical path; xT streams in parallel on the ACT
            # ring.
            hD = D // 2
            tchunks = _chunks(t_cap)
            n0_off, n0 = tchunks[0]
            rings = [nc.sync, nc.scalar]
            # Phase 1 (critical path): weight half A + the first token
            # chunk, alternating rings per k so chunk k of both operands
            # lands at PE consumption rate.
            for k in range(KH):
                rings[k % 2].dma_start(
                    dw_sb[:, k, :hD], dw[k * P : (k + 1) * P, :hD]
                )
                rings[1 - k % 2].dma_start(
                    xT_sb[:, k, :n0], xT[k * P : (k + 1) * P, :n0]
                )
            # Phase 2: weight half B (needed from ~21us).
            for k in range(KH):
                rings[k % 2].dma_start(
                    dw_sb[:, k, hD:], dw[k * P : (k + 1) * P, hD:]
                )
            # Phase 3: remaining token chunks.
            for off, sz in tchunks[1:]:
                for k in range(KH):
                    rings[k % 2].dma_start(
                        xT_sb[:, k, off : off + sz],
                        xT[k * P : (k + 1) * P, off : off + sz],
                    )
            # Phase 4: up weights (needed only when mm2 starts ~75us).
            for k in range(KD):
                rings[k % 2].dma_start(up_sb[:, k, :], up[k * P : (k + 1) * P, :])

            # Warm up the PE clock (HAM un-throttles after ~3.4us of
            # sustained activity) with dummy matmuls that depend on
            # nothing but a memset, so the real matmuls below run at
            # 2.4GHz from the start instead of 1.2GHz.
            warm_sb = const.tile([P, 640], bf16)
            nc.vector.memset(warm_sb[:], 0.0)
            warm_ps = psum.tile([P, 512], f32, tag="ps0", name="warm_ps")
            for i in range(12):
                nc.tensor.matmul(
                    warm_ps[:],
                    warm_sb[:, :P],
                    warm_sb[:, P:640],
                    start=(i == 0),
                    stop=(i == 11),
                )

            # mm1: hidT[D, T] = down^T @ xT with relu, k-outermost over 8
            # concurrent PSUM accumulation groups so each matmul only
            # depends on input chunk k.
            for n_off, n_size in tchunks:
                for mh in range(KD // 8):
                    pss = [
                        psum.tile([P, n_size], f32, tag=f"ps{m}", name=f"ps{m}")
                        for m in range(8)
                    ]
                    for k in range(KH):
                        for m in range(8):
                            md = mh * 8 + m
                            nc.tensor.matmul(
                                pss[m][:],
                                dw_sb[:, k, md * P : (md + 1) * P],
                                xT_sb[:, k, n_off : n_off + n_size],
                                start=(k == 0),
                                stop=(k == KH - 1),
                            )
                    for m in range(8):
                        md = mh * 8 + m
                        nc.vector.tensor_scalar_max(
                            hid_sb[:, md, n_off : n_off + n_size], pss[m][:], 0.0
                        )

            # mm2: yT[H, T] = up^T @ hidT. M runs over H (8 full tiles),
            # the moving dim streams exactly the real token columns.
            gi = 0
            for mh in range(H // P):
                for n_off, n_size in tchunks:
                    ps = psum.tile([P, n_size], f32, tag=f"ps{gi % 8}")
                    gi += 1
                    for k in range(KD):
                        nc.tensor.matmul(
                            ps[:],
                            up_sb[:, k, mh * P : (mh + 1) * P],
                            hid_sb[:, k, n_off : n_off + n_size],
                            start=(k == 0),
                            stop=(k == KD - 1),
                        )
                    yt = outp.tile([P, n_size], f32, tag="yt")
                    nc.vector.tensor_copy(yt[:], ps[:])
                    nc.sync.dma_start(
                        yT[mh * P : (mh + 1) * P, n_off : n_off + n_size], yt[:]
                    )
    nc.compile()
    return nc


def _route(expert_weights, chosen_expert_indices, attention_mask):
    """Host-side routing. Returns (token ids per expert, weights per
    expert, padded positions per (token, k) pair, T_cap)."""
    idx = np.asarray(chosen_expert_indices).reshape(N, K).astype(np.int64)
    wts = np.asarray(expert_weights).reshape(N, K).astype(np.float32)
    mask = np.asarray(attention_mask).reshape(N, 1).astype(np.float32)
    wts = wts * mask

    flat_e = idx.reshape(-1)  # [N*K]
    order = np.argsort(flat_e, kind="stable")
    counts = np.bincount(flat_e, minlength=E)
    offsets = np.zeros(E + 1, np.int64)
    np.cumsum(counts, out=offsets[1:])
    t_cap = max(P, int(counts.max()))

    rank = np.empty(N * K, np.int64)
    rank[order] = np.arange(N * K) - np.repeat(offsets[:-1], counts)
    pad_pos = flat_e * t_cap + rank  # row of pair (n,k) in concat output

    toks = [order[offsets[e] : offsets[e + 1]] // K for e in range(E)]
    w_e = [wts.reshape(-1)[order[offsets[e] : offsets[e + 1]]] for e in range(E)]
    return toks, w_e, pad_pos, t_cap


def kernel(x, attention_mask, expert_weights, chosen_expert_indices, down_proj, up_proj):
    global LAST_RESULT
    xt = np.asarray(x, dtype=np.float32).reshape(N, H)
    toks, w_e, pad_pos, t_cap = _route(
        expert_weights, chosen_expert_indices, attention_mask
    )

    xT_full = np.ascontiguousarray(xt.T)  # [H, N]
    down = np.asarray(down_proj, dtype=np.float32)
    up = np.asarray(up_proj, dtype=np.float32)

    in_maps = []
    for e in range(E):
        t_e = len(toks[e])
        xTg = np.zeros((H, t_cap), dtype=BF16)
        # routing weight folded into the activations (w >= 0, relu is
        # positively homogeneous) so the device output needs no scaling
        xTg[:, :t_e] = (xT_full[:, toks[e]] * w_e[e][None, :]).astype(BF16)
        in_maps.append(
            {"xT": xTg, "dw": down[e].astype(BF16), "up": up[e].astype(BF16)}
        )

    nc = _build_bass(t_cap)
    res = run_bass_kernel_spmd(nc, in_maps, core_ids=list(range(NCORES)))
    LAST_RESULT = res

    # res[e]["yT"] is [H, t_cap]; stack to [E*t_cap, H] token-major.
    y_all = np.concatenate(
        [np.ascontiguousarray(res.results[e]["yT"].T) for e in range(E)], axis=0
    )
    contrib = y_all[pad_pos]  # [N*K, H]
    out = xt + contrib[0::2] + contrib[1::2]
    return out.reshape(B, S, H).astype(np.float32)


# revision 35
# speedup vs baseline: 1.2304x; 1.0025x over previous
"""MoE MLP (top-2 of 8 experts) on 8 Trainium2 NeuronCores.

Strategy: expert parallelism. Each of the 8 cores owns one expert.
Host-side (inside kernel()): route tokens to experts, gather each
expert's tokens into a dense padded [H, T_cap] activation block
(transposed so it is directly usable as the matmul moving operand),
and ship it with that expert's weights to its core. Each core runs
two dense matmuls (down -> relu -> up) entirely out of SBUF and
scales rows by the per-token routing weight. Host-side combine is a
pure gather-add: every token has exactly K=2 expert contributions.

Device compute per core (bf16, fp32 PSUM accumulation):
  hidT[D, T] = down[H, D]^T @ (w * xT)[H, T]   (relu)
  yT[H, T]   = up[D, H]^T @ hidT[D, T]

The routing weight is folded into the gathered activations on the
host: w >= 0 (uniform routing weights times a 0/1 attention mask) and
relu is positively homogeneous, so relu((w*x) @ down) @ up =
w * (relu(x @ down) @ up). That removes the on-device scale pass and
lets both matmuls stream exactly T real token columns with all-full
128-row output tiles.
"""

import os
import sys
import time

import numpy as np

for _p in ("/opt/trn_rl_repo", "/root/.axon_site/_ro/trn_rl_repo"):
    if os.path.isdir(_p) and _p not in sys.path:
        sys.path.append(_p)

import ml_dtypes

import concourse.bass as bass
import concourse.mybir as mybir
from concourse import bacc
from concourse.bass_utils import run_bass_kernel_spmd
from concourse.tile import TileContext

BF16 = ml_dtypes.bfloat16

B, S, H, E, K, D = 1, 4096, 1024, 8, 2, 2048
N = B * S
P = 128
KH = H // P   # 8 contraction tiles for the down matmul
KD = D // P   # 16 contraction tiles for the up matmul
NCORES = 8

# Exposed for test harness introspection (exec_time_ns etc).
LAST_RESULT = None


def _chunks(total: int, maxc: int = 512) -> list[tuple[int, int]]:
    """Equal-ish (offset, size) split of `total` into ceil(total/maxc)
    pieces — keeps every matmul moving-dim well above the dispatch
    floor instead of leaving a tiny remainder chunk."""
    n = -(-total // maxc)
    base, rem = divmod(total, n)
    out, off = [], 0
    for i in range(n):
        sz = base + (1 if i < rem else 0)
        out.append((off, sz))
        off += sz
    return out


def _build_bass(t_cap: int) -> bass.Bass:
    """One expert's MLP: yT[H,T] = up^T @ relu(down^T @ xT)."""
    bf16 = mybir.dt.bfloat16
    f32 = mybir.dt.float32

    nc = bacc.Bacc()
    xT = nc.dram_tensor("xT", [H, t_cap], bf16, kind="ExternalInput")
    dw = nc.dram_tensor("dw", [H, D], bf16, kind="ExternalInput")
    up = nc.dram_tensor("up", [D, H], bf16, kind="ExternalInput")
    yT = nc.dram_tensor("yT", [H, t_cap], f32, kind="ExternalOutput")

    with TileContext(nc) as tc:
        with (
            tc.tile_pool(name="const", bufs=1) as const,
            tc.tile_pool(name="psum", bufs=1, space="PSUM") as psum,
            tc.tile_pool(name="outp", bufs=4) as outp,
        ):
            dw_sb = const.tile([P, KH, D], bf16)
            xT_sb = const.tile([P, KH, t_cap], bf16)
            up_sb = const.tile([P, KD, H], bf16)
            hid_sb = const.tile([P, KD, t_cap], bf16)

            # Per-chunk loads, interleaved so contraction chunk k of both
            # mm1 operands lands together: the k-outer matmul loop below
            # can start as soon as chunk 0 arrives instead of waiting for
            # the full 6.5MB. dw is further split into column halves so
            # the first half of the D tiles (mh=0 groups) can run while
            # the second half is still in flight. up goes on the same
            # ring strictly after dw so it doesn't steal HBM bandwidth
            # from the critical path; xT streams in parallel on the ACT
            # ring.
            hD = D // 2
            tchunks = _chunks(t_cap)
            n0_off, n0 = tchunks[0]
            rings = [nc.sync, nc.scalar]
            # Phase 1 (critical path): weight half A + the first token
            # chunk, alternating rings per k so chunk k of both operands
            # lands at PE consumption rate.
            for k in range(KH):
                rings[k % 2].dma_start(
                    dw_sb[:, k, :hD], dw[k * P : (k + 1) * P, :hD]
                )
                rings[1 - k % 2].dma_start(
                    xT_sb[:, k, :n0], xT[k * P : (k + 1) * P, :n0]
                )
            # Phase 2: weight half B (needed from ~21us).
            for k in range(KH):
                rings[k % 2].dma_start(
                    dw_sb[:, k, hD:], dw[k * P : (k + 1) * P, hD:]
                )
            # Phase 3: remaining token chunks.
            for off, sz in tchunks[1:]:
                for k in range(KH):
                    rings[k % 2].dma_start(
                        xT_sb[:, k, off : off + sz],
                        xT[k * P : (k + 1) * P, off : off + sz],
                    )
            # Phase 4: up weights (needed only when mm2 starts ~75us).
            for k in range(KD):
                rings[k % 2].dma_start(up_sb[:, k, :], up[k * P : (k + 1) * P, :])

            # Warm up the PE clock (HAM un-throttles after ~3.4us of
            # sustained activity) with dummy matmuls that depend on
            # nothing but a memset, so the real matmuls below run at
            # 2.4GHz from the start instead of 1.2GHz.
            warm_sb = const.tile([P, 640], bf16)
            nc.vector.memset(warm_sb[:], 0.0)
            warm_ps = psum.tile([P, 512], f32, tag="ps0", name="warm_ps")
            for i in range(12):
                nc.tensor.matmul(
                    warm_ps[:],
                    warm_sb[:, :P],
                    warm_sb[:, P:640],
                    start=(i == 0),
                    stop=(i == 11),
                )

            # mm1: hidT[D, T] = down^T @ xT with relu, k-outermost over 8
            # concurrent PSUM accumulation groups so each matmul only
            # depends on input chunk k.
            for n_off, n_size in tchunks:
                for mh in range(KD // 8):
                    pss = [
                        psum.tile([P, n_size], f32, tag=f"ps{m}", name=f"ps{m}")
                        for m in range(8)
                    ]
                    for k in range(KH):
                        for m in range(8):
                            md = mh * 8 + m
                            nc.tensor.matmul(
                                pss[m][:],
                                dw_sb[:, k, md * P : (md + 1) * P],
                                xT_sb[:, k, n_off : n_off + n_size],
                                start=(k == 0),
                                stop=(k == KH - 1),
                            )
                    for m in range(8):
                        md = mh * 8 + m
                        nc.vector.tensor_scalar_max(
                            hid_sb[:, md, n_off : n_off + n_size], pss[m][:], 0.0
                        )

            # mm2: yT[H, T] = up^T @ hidT. M runs over H (8 full tiles),
            # the moving dim streams exactly the real token columns.
            gi = 0
            for mh in range(H // P):
                for n_off, n_size in tchunks:
                    ps = psum.tile([P, n_size], f32, tag=f"ps{gi % 8}")
                    gi += 1
                    for k in range(KD):
                        nc.tensor.matmul(
                            ps[:],
                            up_sb[:, k, mh * P : (mh + 1) * P],
                            hid_sb[:, k, n_off : n_off + n_size],
                            start=(k == 0),
                            stop=(k == KD - 1),
                        )
                    yt = outp.tile([P, n_size], f32, tag="yt")
                    nc.vector.tensor_copy(yt[:], ps[:])
                    nc.sync.dma_start(
                        yT[mh * P : (mh + 1) * P, n_off : n_off + n_size], yt[:]
                    )
    nc.compile()
    return nc


def _route(expert_weights, chosen_expert_indices, attention_mask):
    """Host-side routing. Returns (token ids per expert, weights per
    expert, padded positions per (token, k) pair, T_cap)."""
    idx = np.asarray(chosen_expert_indices).reshape(N, K).astype(np.int64)
    wts = np.asarray(expert_weights).reshape(N, K).astype(np.float32)
    mask = np.asarray(attention_mask).reshape(N, 1).astype(np.float32)
    wts = wts * mask

    flat_e = idx.reshape(-1)  # [N*K]
    order = np.argsort(flat_e, kind="stable")
    counts = np.bincount(flat_e, minlength=E)
    offsets = np.zeros(E + 1, np.int64)
    np.cumsum(counts, out=offsets[1:])
    t_cap = max(P, int(counts.max()))

    rank = np.empty(N * K, np.int64)
    rank[order] = np.arange(N * K) - np.repeat(offsets[:-1], counts)
    pad_pos = flat_e * t_cap + rank  # row of pair (n,k) in concat output

    toks = [order[offsets[e] : offsets[e + 1]] // K for e in range(E)]
    w_e = [wts.reshape(-1)[order[offsets[e] : offsets[e + 1]]] for e in range(E)]
    return toks, w_e, pad_pos, t_cap


def kernel(x, attention_mask, expert_weights, chosen_expert_indices, down_proj, up_proj):
    global LAST_RESULT
    xt = np.asarray(x, dtype=np.float32).reshape(N, H)
    toks, w_e, pad_pos, t_cap = _route(
        expert_weights, chosen_expert_indices, attention_mask
    )

    xT_full = np.ascontiguousarray(xt.T)  # [H, N]
    down = np.asarray(down_proj, dtype=np.float32)
    up = np.asarray(up_proj, dtype=np.float32)

    in_maps = []
    for e in range(E):
        t_e = len(toks[e])
        xTg = np.zeros((H, t_cap), dtype=BF16)
        # routing weight folded into the activations (w >= 0, relu is
        # positively homogeneous) so the device output needs no scaling
        xTg[:, :t_e] = (xT_full[:, toks[e]] * w_e[e][None, :]).astype(BF16)
        in_maps.append(
            {"xT": xTg, "dw": down[e].astype(BF16), "up": up[e].astype(BF16)}
        )

    nc = _build_bass(t_cap)
    # First execution of a freshly loaded NEFF occasionally fails with a
    # transient NRT_EXEC_UNIT_UNRECOVERABLE; a retry has always succeeded.
    last_err = None
    for attempt in range(3):
        try:
            res = run_bass_kernel_spmd(nc, in_maps, core_ids=list(range(NCORES)))
            break
        except Exception as e:  # noqa: BLE001
            last_err = e
            time.sleep(3.0)
    else:
        raise last_err
    LAST_RESULT = res

    # res[e]["yT"] is [H, t_cap]; stack to [E*t_cap, H] token-major.
    y_all = np.concatenate(
        [np.ascontiguousarray(res.results[e]["yT"].T) for e in range(E)], axis=0
    )
    contrib = y_all[pad_pos]  # [N*K, H]
    out = xt + contrib[0::2] + contrib[1::2]
    return out.reshape(B, S, H).astype(np.float32)
